# revision 2
# baseline (speedup 1.0000x reference)
"""Trainium2 Bass kernel for nn_CrossAttentionBlock (GroupNorm + 1x1-conv Q +
cross-attention over cond + output projection + residual).

Full-input contract: kernel(**inputs) takes the complete unsharded inputs and
returns the full [16, 512, 64, 64] float32 output.  Internally shards
data-parallel over batch across 8 NeuronCores (2 batches per core), runs one
SPMD Bass/Tile kernel via run_bass_kernel_spmd, and concatenates the results.

Layout strategy (per core, per batch, channels-first [C, HW] everywhere):
  x [512, 4096] resident in SBUF -> groupnorm stats via DVE bn_stats/bn_aggr
  (one pass over x), group 16-chan reduce/scatter via tiny indicator matmuls
  -> per-channel scale/bias -> per-hw-chunk (512 cols):
    xn = GpSimd tensor_scalar(x*sc + tc) in bf16       (Pool engine, SBUF only)
    q  = qwT.T @ xn; PSUM->SBUF cast + q_b bias on ACT  [C, 512]  PE
    per head h (hd=64): logits^T = kT_h.T @ q_h  [77, 512] PE (head pairs at
      partition bases 0/64 -> concurrent PE row-groups)
    exp via ACT -> per-head [77, 512] bf16 (no max subtraction; logits O(10))
    sums_h = ones77.T @ exp_h replicated over the head's 64 out channels (PE)
    rcp = DVE reciprocal_approx_fast (NOT nc.vector.reciprocal: that one is a
      bit-exact iterative divide at ~6 cycles/elem on HW)
    av = v_h @ exp_h [64, 512] PE pair-packed; normalize = DVE tensor_mul
    out = pwT.T @ av_norm; residual + proj bias fused in one DVE
      scalar_tensor_tensor: osb = (psum + pb_col) + x_resident
    one coalesced DMA per chunk stores [128, 4, 512] -> out[b, :, cs]
Weights are transposed/cast to bf16 on the host (layout prep only).
"""

import sys

for _p in ("/opt/trn_rl_repo",):
    if _p not in sys.path:
        sys.path.append(_p)

from contextlib import ExitStack

import numpy as np
import ml_dtypes

import concourse.bacc as bacc
import concourse.tile as tile
from concourse import mybir
from concourse.bass_utils import run_bass_kernel_spmd

BF16 = ml_dtypes.bfloat16

N_CORES = 8
B, C, H, W = 16, 512, 64, 64
HW = H * W                      # 4096
L, CD = 77, 768
NH, HD = 8, 64                  # heads, head dim
NG, GS = 32, 16                 # groups, channels per group
EPS = 1e-6
B_LOC = B // N_CORES            # 2
NT = C // 128                   # 4 channel tiles
KT = CD // 128                  # 6 cond-dim tiles
CH = 512                        # hw chunk
NCH = HW // CH                  # 8
GPT = 128 // GS                 # 8 groups per 128-channel tile


def _build_nc(nch=NCH, reps=1):
    f32 = mybir.dt.float32
    bf16 = mybir.dt.bfloat16
    nc = bacc.Bacc("TRN2", target_bir_lowering=False, debug=False)

    x_d = nc.dram_tensor("x", [B_LOC, C, HW], f32, kind="ExternalInput").ap()
    condT_d = nc.dram_tensor("condT", [B_LOC, CD, L], bf16, kind="ExternalInput").ap()
    qwT_d = nc.dram_tensor("qwT", [C, C], bf16, kind="ExternalInput").ap()
    kwT_d = nc.dram_tensor("kwT", [CD, C], bf16, kind="ExternalInput").ap()
    vwT_d = nc.dram_tensor("vwT", [CD, C], bf16, kind="ExternalInput").ap()
    pwT_d = nc.dram_tensor("pwT", [C, C], bf16, kind="ExternalInput").ap()
    gamma_d = nc.dram_tensor("gamma", [C, 1], f32, kind="ExternalInput").ap()
    beta_d = nc.dram_tensor("beta", [C, 1], f32, kind="ExternalInput").ap()
    qb_d = nc.dram_tensor("qb", [C, 1], f32, kind="ExternalInput").ap()
    kb_d = nc.dram_tensor("kb", [C, 1], f32, kind="ExternalInput").ap()
    vb_d = nc.dram_tensor("vb", [1, C], f32, kind="ExternalInput").ap()
    pb_d = nc.dram_tensor("pb", [C, 1], f32, kind="ExternalInput").ap()
    scale_d = nc.dram_tensor("scale", [1, 1], f32, kind="ExternalInput").ap()
    g16_d = nc.dram_tensor("g16", [128, GPT], f32, kind="ExternalInput").ap()
    g16T_d = nc.dram_tensor("g16T", [GPT, 128], f32, kind="ExternalInput").ap()
    out_d = nc.dram_tensor("out", [B_LOC, C, HW], f32, kind="ExternalOutput").ap()

    AO = mybir.AluOpType
    AF = mybir.ActivationFunctionType

    with tile.TileContext(nc) as tc, ExitStack() as ctx:
        # --- pools ---
        wp = ctx.enter_context(tc.tile_pool(name="weights", bufs=1))
        sb1 = ctx.enter_context(tc.tile_pool(name="work1", bufs=1))
        sb2 = ctx.enter_context(tc.tile_pool(name="work2", bufs=2))
        sb3 = ctx.enter_context(tc.tile_pool(name="work3", bufs=2))
        # PSUM: q(2) + at(2, shared qk/av) + sums(2) + o(2) = 8 banks
        ps_q = ctx.enter_context(tc.tile_pool(name="ps_q", bufs=2, space="PSUM"))
        ps_at = ctx.enter_context(tc.tile_pool(name="ps_at", bufs=2, space="PSUM"))
        ps_sm = ctx.enter_context(tc.tile_pool(name="ps_sm", bufs=2, space="PSUM"))
        ps_o = ctx.enter_context(tc.tile_pool(name="ps_o", bufs=2, space="PSUM"))

        # --- load persistent weights/constants ---
        qwT = [wp.tile([128, C], bf16, tag=f"qwT{j}", name=f"qwT{j}")
               for j in range(NT)]
        pwT = [wp.tile([128, C], bf16, tag=f"pwT{j}", name=f"pwT{j}")
               for j in range(NT)]
        kwT = [wp.tile([128, C], bf16, tag=f"kwT{j}", name=f"kwT{j}")
               for j in range(KT)]
        vwT = [wp.tile([128, C], bf16, tag=f"vwT{j}", name=f"vwT{j}")
               for j in range(KT)]
        for j in range(NT):
            nc.sync.dma_start(qwT[j][:], qwT_d[128 * j:128 * (j + 1), :])
            nc.sync.dma_start(pwT[j][:], pwT_d[128 * j:128 * (j + 1), :])
        for j in range(KT):
            nc.sync.dma_start(kwT[j][:], kwT_d[128 * j:128 * (j + 1), :])
            nc.sync.dma_start(vwT[j][:], vwT_d[128 * j:128 * (j + 1), :])

        g16 = wp.tile([128, GPT], f32, tag="g16")
        nc.sync.dma_start(g16[:], g16_d[:, :])
        g16T = wp.tile([GPT, 128], f32, tag="g16T")
        nc.sync.dma_start(g16T[:], g16T_d[:, :])

        # column vectors: 0-3 gamma, 4-7 beta, 8-11 qb, 12-15 kb, 16-19 pb
        colv = wp.tile([128, 20], f32, tag="colv")
        for t in range(NT):
            s = slice(128 * t, 128 * (t + 1))
            nc.sync.dma_start(colv[:, t:t + 1], gamma_d[s, :])
            nc.sync.dma_start(colv[:, 4 + t:5 + t], beta_d[s, :])
            nc.sync.dma_start(colv[:, 8 + t:9 + t], qb_d[s, :])
            nc.sync.dma_start(colv[:, 12 + t:13 + t], kb_d[s, :])
            nc.sync.dma_start(colv[:, 16 + t:17 + t], pb_d[s, :])
        vb_row = wp.tile([1, C], f32, tag="vb_row")
        nc.sync.dma_start(vb_row[:], vb_d[:, :])
        s11 = wp.tile([1, 1], f32, tag="s11")
        nc.sync.dma_start(s11[:], scale_d[:, :])
        scale_col = wp.tile([128, 1], f32, tag="scale_col")
        nc.gpsimd.partition_broadcast(scale_col[:], s11[:])
        ones77 = wp.tile([L, 64], bf16, tag="ones77")
        nc.gpsimd.memset(ones77[:], 1.0)
        eps_col = wp.tile([GPT, 1], f32, tag="eps_col")
        nc.gpsimd.memset(eps_col[:], EPS)
        # k bias pre-scaled by `scale` (folded into the kT copy)
        kbs = wp.tile([128, NT], f32, tag="kbs")
        for t in range(NT):
            nc.vector.tensor_mul(kbs[:, t:t + 1], colv[:, 12 + t:13 + t],
                                 scale_col[:])
        # v bias broadcast over the 77 cond rows (batch independent)
        vb_bc = wp.tile([L, C], f32, tag="vb_bc")
        nc.gpsimd.partition_broadcast(vb_bc[:], vb_row[:])

        rep_ctx = tc.For_i(0, reps, 1) if reps > 1 else None
        if rep_ctx is not None:
            rep_ctx.__enter__()
        for b in range(B_LOC):
            # ---------- load x (per-tile DMAs so stats can stream) ----------
            xball = sb1.tile([128, NT, HW], f32, tag="x", name="x")
            for t in range(NT):
                nc.sync.dma_start(xball[:, t, :],
                                  x_d[b, 128 * t:128 * (t + 1), :])

            # ---------- groupnorm stats: one DVE pass via bn_stats ----------
            bnout = sb2.tile([128, NT, 8, 6], f32, tag="bnout")
            for t in range(NT):
                for j in range(NCH):
                    nc.vector.bn_stats(bnout[:, t, j, :],
                                       xball[:, t, CH * j:CH * (j + 1)])
            # per-channel mean/var -> gin cols 2t = mean, 2t+1 = E[x^2]
            gin = sb2.tile([128, 2 * NT], f32, tag="gin")
            tmp4 = sb2.tile([128, NT], f32, tag="tmp4")
            for t in range(NT):
                nc.vector.bn_aggr(gin[:, 2 * t:2 * t + 2], bnout[:, t, :, :])
            nc.vector.tensor_mul(tmp4[:], gin[:, 0:2 * NT:2], gin[:, 0:2 * NT:2])
            nc.vector.tensor_add(gin[:, 1:2 * NT:2], gin[:, 1:2 * NT:2], tmp4[:])
            # group reduce (sum over each 16-channel group) in one matmul
            gst = ps_sm.tile([GPT, 2 * NT], f32, tag="sm")
            nc.tensor.matmul(gst[:], g16[:], gin[:], start=True, stop=True)
            # per-group mu / rsig
            gw = sb2.tile([GPT, 2 * NT], f32, tag="gw")
            tmpg = sb2.tile([GPT, NT], f32, tag="tmpg")
            nc.vector.tensor_scalar_mul(gw[:], gst[:], 1.0 / GS)
            nc.vector.tensor_mul(tmpg[:], gw[:, 0:2 * NT:2], gw[:, 0:2 * NT:2])
            nc.vector.tensor_sub(gw[:, 1:2 * NT:2], gw[:, 1:2 * NT:2], tmpg[:])
            tmpg2 = sb2.tile([GPT, NT], f32, tag="tmpg2")
            nc.scalar.activation(tmpg2[:], gw[:, 1:2 * NT:2], AF.Sqrt,
                                 bias=eps_col[:])
            nc.vector.reciprocal(gw[:, 1:2 * NT:2], tmpg2[:])
            # scatter groups -> channels in one matmul: cols 2t=mu, 2t+1=rsig
            cst = ps_sm.tile([128, 2 * NT], f32, tag="sm")
            nc.tensor.matmul(cst[:], g16T[:], gw[:], start=True, stop=True)
            # per-channel scale/bias: sc = gamma*rsig, tc = beta - mu*sc
            scb = sb2.tile([128, 2 * NT], f32, tag="scb")
            tmpc = sb2.tile([128, NT], f32, tag="tmpc")
            nc.vector.tensor_mul(scb[:, 0:2 * NT:2], colv[:, 0:NT],
                                 cst[:, 1:2 * NT:2])
            nc.vector.tensor_mul(tmpc[:], cst[:, 0:2 * NT:2],
                                 scb[:, 0:2 * NT:2])
            nc.vector.tensor_sub(scb[:, 1:2 * NT:2], colv[:, NT:2 * NT],
                                 tmpc[:])

            # ---------- K^T and V projections from cond ----------
            cT = [sb2.tile([128, L], bf16, tag=f"cT{j}", name=f"cT{j}")
                  for j in range(KT)]
            for j in range(KT):
                nc.sync.dma_start(cT[j][:], condT_d[b, 128 * j:128 * (j + 1), :])
            kT = [sb2.tile([128, L], bf16, tag=f"kT{t}", name=f"kT{t}")
                  for t in range(NT)]
            v_sb = sb2.tile([L, C], bf16, tag="v_sb")
            for t in range(NT):
                cs = slice(128 * t, 128 * (t + 1))
                pk = ps_q.tile([128, CH], f32, tag="q")
                for j in range(KT):
                    nc.tensor.matmul(pk[:, 0:L], kwT[j][:, cs], cT[j][:],
                                     start=(j == 0), stop=(j == KT - 1))
                nc.scalar.activation(kT[t][:], pk[:, 0:L], AF.Identity,
                                     bias=kbs[:, t:t + 1], scale=scale_col[:])
                pv = ps_at.tile([128, CH], f32, tag="at")
                for j in range(KT):
                    nc.tensor.matmul(pv[0:L, 0:128], cT[j][:], vwT[j][:, cs],
                                     start=(j == 0), stop=(j == KT - 1))
                nc.vector.tensor_add(v_sb[:, cs], pv[0:L, 0:128], vb_bc[:, cs])

            # ---------- hw-chunk pipeline ----------
            for cix in range(nch):
                cs = slice(CH * cix, CH * (cix + 1))
                # groupnorm apply on GpSimd (bf16 out)
                xn = [sb2.tile([128, CH], bf16, tag=f"xn{t}", name=f"xn{t}")
                      for t in range(NT)]
                for t in range(NT):
                    nc.gpsimd.tensor_scalar(xn[t][:], xball[:, t, cs],
                                            scb[:, 2 * t:2 * t + 1],
                                            scb[:, 2 * t + 1:2 * t + 2],
                                            op0=AO.mult, op1=AO.add)
                # q projection; bias fused into the ACT PSUM->SBUF cast
                q_sb = [sb2.tile([128, CH], bf16, tag=f"q{m}", name=f"qsb{m}")
                        for m in range(NT)]
                for m in range(NT):
                    ms = slice(128 * m, 128 * (m + 1))
                    pq = ps_q.tile([128, CH], f32, tag="q")
                    for k in range(NT):
                        nc.tensor.matmul(pq[:], qwT[k][:, ms], xn[k][:],
                                         start=(k == 0), stop=(k == NT - 1))
                    nc.scalar.activation(q_sb[m][:], pq[:], AF.Identity,
                                         bias=colv[:, 8 + m:9 + m])
                # attention: per-head logits^T -> exp (bf16)
                eh = [sb2.tile([L, CH], bf16, tag=f"eh{h}", name=f"eh{h}")
                      for h in range(NH)]
                for h in range(NH):
                    t_, off = h // 2, 64 * (h % 2)
                    pqk = ps_at.tile([128, CH], f32, tag="at")
                    nc.tensor.matmul(pqk[0:L, :], kT[t_][off:off + 64, :],
                                     q_sb[t_][off:off + 64, :],
                                     start=True, stop=True)
                    nc.scalar.activation(eh[h][:], pqk[0:L, :], AF.Exp)
                # AV (pair-packed) + PE-replicated sums + normalize
                prj = [sb2.tile([128, CH], bf16, tag=f"pi{p}", name=f"pi{p}")
                       for p in range(NT)]
                for p in range(NT):
                    psm = ps_sm.tile([128, CH], f32, tag="sm")
                    pav = ps_at.tile([128, CH], f32, tag="at")
                    for h in (2 * p, 2 * p + 1):
                        off = 64 * (h % 2)
                        # sum of exp replicated over this head's 64 rows
                        nc.tensor.matmul(psm[off:off + 64, :], ones77[:],
                                         eh[h][:], start=True, stop=True)
                        nc.tensor.matmul(pav[off:off + 64, :],
                                         v_sb[:, 64 * h:64 * h + 64], eh[h][:],
                                         start=True, stop=True)
                    rcp = sb2.tile([128, CH], f32, tag=f"rcp{p % 2}",
                                   name=f"rcp{p}")
                    nc.vector.reciprocal_approx_fast(rcp[:], psm[:])
                    nc.vector.tensor_mul(prj[p][:], pav[:], rcp[:])
                # output projection; bias+residual fused in one DVE op;
                # one coalesced store for the whole chunk
                osb = sb3.tile([128, NT, CH], f32, tag="osb", name="osb")
                for m in range(NT):
                    po = ps_o.tile([128, CH], f32, tag="o")
                    for k in range(NT):
                        nc.tensor.matmul(po[:], pwT[k][:, 128 * m:128 * (m + 1)],
                                         prj[k][:],
                                         start=(k == 0), stop=(k == NT - 1))
                    nc.vector.scalar_tensor_tensor(
                        osb[:, m, :], po[:], colv[:, 16 + m:17 + m],
                        xball[:, m, cs], op0=AO.add, op1=AO.add)
                out_view = out_d[b, :, cs].rearrange("(m p) w -> p m w", p=128)
                nc.sync.dma_start(out_view, osb[:])
        if rep_ctx is not None:
            rep_ctx.__exit__(None, None, None)

    nc.compile()
    return nc


_NC_CACHE = None


def _get_nc():
    global _NC_CACHE
    if _NC_CACHE is None:
        _NC_CACHE = _build_nc()
    return _NC_CACHE


def make_in_maps(x, cond, gamma, beta, q_w, q_b, k_w, k_b, v_w, v_b,
                 proj_w, proj_b, scale):
    x = np.asarray(x, np.float32).reshape(B, C, HW)
    condT = np.asarray(cond, np.float32).transpose(0, 2, 1).astype(BF16)
    qwT = np.ascontiguousarray(np.asarray(q_w, np.float32).T).astype(BF16)
    kwT = np.ascontiguousarray(np.asarray(k_w, np.float32).T).astype(BF16)
    vwT = np.ascontiguousarray(np.asarray(v_w, np.float32).T).astype(BF16)
    pwT = np.ascontiguousarray(np.asarray(proj_w, np.float32).T).astype(BF16)
    g16 = np.zeros((128, GPT), np.float32)
    for p in range(128):
        g16[p, p // GS] = 1
    g16T = np.ascontiguousarray(g16.T)
    com = dict(
        qwT=qwT, kwT=kwT, vwT=vwT, pwT=pwT,
        gamma=np.asarray(gamma, np.float32).reshape(C, 1),
        beta=np.asarray(beta, np.float32).reshape(C, 1),
        qb=np.asarray(q_b, np.float32).reshape(C, 1),
        kb=np.asarray(k_b, np.float32).reshape(C, 1),
        vb=np.asarray(v_b, np.float32).reshape(1, C),
        pb=np.asarray(proj_b, np.float32).reshape(C, 1),
        scale=np.asarray(scale, np.float32).reshape(1, 1),
        g16=g16, g16T=g16T,
    )
    in_maps = []
    for cix in range(N_CORES):
        bs = slice(B_LOC * cix, B_LOC * (cix + 1))
        m = dict(com)
        m["x"] = np.ascontiguousarray(x[bs])
        m["condT"] = np.ascontiguousarray(condT[bs])
        in_maps.append(m)
    return in_maps


def kernel(x, cond, gamma, beta, q_w, q_b, k_w, k_b, v_w, v_b,
           proj_w, proj_b, scale):
    nc = _get_nc()
    in_maps = make_in_maps(x, cond, gamma, beta, q_w, q_b, k_w, k_b,
                           v_w, v_b, proj_w, proj_b, scale)
    res = run_bass_kernel_spmd(nc, in_maps, core_ids=list(range(N_CORES)))
    out = np.concatenate([r["out"] for r in res.results], axis=0)
    return out.reshape(B, C, H, W).astype(np.float32)
